# revision 36
# baseline (speedup 1.0000x reference)
"""Trainium2 Bass kernel for nn_AttentionScore (sparse local attention scores).

Reference computation (B=4, C=64, N=16384, S=16):
    tmp   = xyz[:, :, :, None] - neighbor_xyz            # [B,3,N,S]
    pos   = concat([tmp, ||tmp||], axis=1)               # [B,4,N,S]
    k     = Wk @ (neighbor_points + Wpos @ pos + bpos)   # [B,C,N,S]
    attn  = softmax_s((points*scale) . k)                # [B,N,S]

Softmax over s is shift-invariant, so every term constant in s drops out:
    attn[m,s] ~ sum_c qW[c,m]*np[c,m,s] - sum_j qp[j,m]*nx[j,m,s]
                + qp3[m]*sqrt(max(0, ||xyz||^2 + sum_j (nx[j]-2*xyz[j])*nx[j]))
with qW = (scale*Wk)^T @ points, qp = ((scale*Wk)@Wpos)^T @ points
(bpos and all xyz-only dot products cancel).

All bulk tensors are pre-cast to bf16 on the host (tolerance is 2e-2;
measured end-to-end error of the bf16 pipeline is ~3e-3), halving HBM
traffic and enabling the DVE 2x bf16 tensor_tensor mode.

Sharding: N split contiguously across 8 cores (no communication).
m = b*2048 + n_local in [0, 8192) per core, halves h = m // 4096.

Main-term dataflow per core, per supertile t (8 supertiles, 512 m per half):
  np staged bf16 as [128 part = (h,c), cols (mh:8, s:16, ml:64)];
  DVE multiplies by qW broadcast over s (dense innermost ml run keeps the
  2x bf16 mode); TensorE reduces the 64 c-partitions per half with
  selector-column matmuls accumulating into one [32, 512] PSUM tile
  (row h*16 + (mh*2+sh)); ScalarE copies PSUM->SBUF; a partition-scatter
  SBUF->SBUF DMA lands rows in the softmax layout [p=m//64, (s:16, mi:64)].
"""

import sys

sys.path.insert(0, "/opt/trn_rl_repo")

import numpy as np
import ml_dtypes

import concourse.bass as bass
import concourse.bacc as bacc
import concourse.tile as tile
from concourse import mybir
from concourse.bass_utils import run_bass_kernel_spmd

F32 = mybir.dt.float32
BF16 = mybir.dt.bfloat16
AF = mybir.ActivationFunctionType
AX = mybir.AxisListType
NPBF = ml_dtypes.bfloat16

B, C, N, S = 4, 64, 16384, 16
NCORES = 8
NL = N // NCORES            # 2048 points per core
M = B * NL                  # 8192 (b, n) rows per core
MH = M // 2                 # 4096 rows per half
NT = 8                      # supertiles
MB = MH // NT               # 512 m per half per supertile
SCALE = float(C) ** -0.5

# per-supertile NP tile: [128 p=(h,c), cols (mh:8, s:16, ml:64)] = 8192 cols
TS = MB * S                 # 8192 columns per supertile


def _body(tc):
    nc = tc.nc
    dma = nc.sync.dma_start
    gdma = nc.gpsimd.dma_start

    NP = nc.dram_tensor("NP", [128, NT * TS], BF16, kind="ExternalInput").ap()
    NX = nc.dram_tensor("NX", [128, 3 * S * 64], BF16, kind="ExternalInput").ap()
    XYZ = nc.dram_tensor("XYZ", [128, 3 * 64], F32, kind="ExternalInput").ap()
    # points packed full-width: row u*64+c holds q[c, u*MH + mm]
    P = nc.dram_tensor("P", [128, MH], BF16, kind="ExternalInput").ap()
    # all weights in one transfer: cols 0:64 Wk (dup on both partition
    # halves), 64:128 Wk^T (rows 0:64 valid), 128:132 Wpos (rows 0:64 valid)
    WA = nc.dram_tensor("WA", [128, C + C + 4], F32, kind="ExternalInput").ap()
    OUT = nc.dram_tensor("OUT", [128, 64 * S], BF16, kind="ExternalOutput").ap()

    with (
        tc.tile_pool(name="const", bufs=1) as cp,
        tc.tile_pool(name="small", bufs=1) as sp,
        tc.tile_pool(name="work", bufs=2) as wkp_,
        tc.tile_pool(name="npt", bufs=4) as npp,
        tc.tile_pool(name="prod", bufs=2) as prp,
        tc.tile_pool(name="sc", bufs=2) as scp,
        tc.tile_pool(name="psm", bufs=2, space="PSUM") as psm,
        tc.tile_pool(name="psq", bufs=2, space="PSUM") as psq,
    ):
        # ---- loads: weights + first P slice on sync ahead of the NP
        # stream; P tail + NX/XYZ on gpsimd (runs concurrently) ----
        wa = cp.tile([128, C + C + 4], F32)
        dma(wa[:], WA)
        p2 = cp.tile([128, MH], BF16)
        # cc=0 slice on sync so chunk 0 unblocks early; tail via gpsimd
        dma(p2[:, 0:512], P[:, 0:512])

        npts = {}

        def issue_np(t):
            npt = npp.tile([128, TS], BF16, name="npt")
            if t in (0, NT - 1):
                # halved so the first/last supertile can start on 1 MB
                dma(npt[:, 0:TS // 2], NP[:, t * TS:t * TS + TS // 2])
                dma(npt[:, TS // 2:TS], NP[:, t * TS + TS // 2:(t + 1) * TS])
            else:
                dma(npt[:], NP[:, t * TS:(t + 1) * TS])
            npts[t] = npt

        # sync-queue slotting: every sizeable transfer rides this one queue
        # so nothing splits HBM bandwidth with the NP stream
        issue_np(0)
        dma(p2[:, 512:MH], P[:, 512:MH])
        issue_np(1)
        nxt = cp.tile([128, 3 * S * 64], BF16)
        dma(nxt[:], NX)
        issue_np(2)
        issue_np(3)
        xyzt = cp.tile([128, 3 * 64], F32)
        gdma(xyzt[:], XYZ)

        # block-diagonal scaled weights: one matmul covers both halves.
        # wkd[u*64+c, h*64+c'] = sWk[c, c'] if u == h else 0.
        wkd = sp.tile([128, 128], BF16)
        nc.vector.memset(wkd[:], 0.0)
        nc.vector.tensor_scalar_mul(wkd[0:64, 0:64], wa[0:64, 0:C], SCALE)
        nc.vector.tensor_scalar_mul(wkd[64:128, 64:128], wa[64:128, 0:C], SCALE)
        # wkts2 = [sWk^T | sWk^T]: lhsT whose 128 out rows duplicate Wkp
        wkts2 = sp.tile([C, 2 * C], F32)
        nc.vector.tensor_scalar_mul(wkts2[:, 0:C], wa[0:64, C:2 * C], SCALE)
        nc.vector.tensor_scalar_mul(wkts2[:, C:2 * C], wa[0:64, C:2 * C], SCALE)
        wp0 = wa[0:64, 2 * C:2 * C + 4]

        # Selector for the channel-reduce matmuls: chunk k uses cols
        # [k*32, (k+1)*32); col h*16+k is 1 on the half-h partitions, so
        # chunk k's half-h sum lands on PSUM row h*16+k.
        hs = sp.tile([128, 16 * 32], BF16)
        nc.vector.memset(hs[:], 0.0)
        for k in range(16):
            nc.vector.memset(hs[0:64, k * 32 + k:k * 32 + k + 1], 1.0)
            nc.vector.memset(hs[64:128, k * 32 + 16 + k:k * 32 + 16 + k + 1], 1.0)

        qw = cp.tile([128, MH], BF16)        # row h*64+c: qW[c, h*MH + mm]
        qpt = cp.tile([128, 4 * 64], BF16)   # [p=m//64, (j:4, mi:64)]
        attn1 = cp.tile([128, 64 * S], F32)  # [p=m//64, (s:16, mi:64)]
        attn2 = cp.tile([128, 64 * S], F32)

        def qw_chunk(cc):
            # qW chunk cc, both halves in one block-diagonal matmul:
            # rows h*64+c' = qW[c', h*MH + chunk cc]
            pq = psq.tile([128, 512], F32, name="pq")
            nc.tensor.matmul(
                pq[:],
                lhsT=wkd[:],
                rhs=p2[:, cc * 512:(cc + 1) * 512],
                start=True,
                stop=True,
            )
            nc.scalar.copy(qw[:, cc * 512:(cc + 1) * 512], pq[:])

        # chunks 0/1 ahead of everything so supertile 0 can start the
        # moment NP tile 0 lands
        qw_chunk(0)
        qw_chunk(1)

        # ---- phase 1a (up front): qp = ((sWk)Wpos)^T q for all chunks, so
        # qpt is complete before phase 2.  Remaining qW chunks are
        # interleaved with the supertile loop below.
        with (
            tc.tile_pool(name="qps_p", bufs=2) as qpsp,
            tc.tile_pool(name="psp", bufs=3, space="PSUM") as psp,
            tc.tile_pool(name="psw", bufs=1, space="PSUM") as psw,
        ):
            # Wkp[c, j] = sum_c' sWk[c, c'] Wpos[c', j], duplicated onto
            # both partition halves via the widened lhsT (fp32, tiny)
            pwkp = psw.tile([128, 4], F32)
            nc.tensor.matmul(pwkp[:], lhsT=wkts2[:], rhs=wp0, start=True, stop=True)
            # block-diagonal Wkp: col h*4+j = Wkp[:, j] on half-h rows
            wkpd = sp.tile([128, 8], BF16)
            nc.vector.memset(wkpd[:], 0.0)
            nc.scalar.copy(wkpd[0:64, 0:4], pwkp[0:64, :])
            nc.scalar.copy(wkpd[64:128, 4:8], pwkp[64:128, :])

            qps_tiles = {}
            for cc in range(8):
                # one matmul per chunk: rows (h, j) = qp[j, h*MH + chunk]
                pp = psp.tile([8, 512], F32)
                nc.tensor.matmul(
                    pp[:],
                    lhsT=wkpd[:],
                    rhs=p2[:, cc * 512:(cc + 1) * 512],
                    start=True,
                    stop=True,
                )
                gg = cc // 4
                if gg not in qps_tiles:
                    qps_tiles[gg] = qpsp.tile([8, 2048], BF16, name="qps", tag="qps")
                qps = qps_tiles[gg]
                nc.scalar.copy(qps[:, (cc % 4) * 512:(cc % 4 + 1) * 512], pp[:])
                if cc % 4 == 3:
                    # scatter into softmax layout: qpt[p, j*64+mi]; rows
                    # (h, j) of qps go to partition group g = h*2 + gg
                    for h in range(2):
                        g = h * 2 + gg
                        for j in range(4):
                            gdma(
                                qpt[g * 32:(g + 1) * 32, j * 64:(j + 1) * 64],
                                qps[h * 4 + j:h * 4 + j + 1, :],
                            )
                    del qps_tiles[gg]

        # phase 2 (positional term), split into three pieces that slot into
        # the DVE gaps between supertile muls
        nx4 = nxt[:].rearrange("p (j s mi) -> p j s mi", j=3, s=S, mi=64)
        p2state = {}

        def phase2a():
            # xyz-derived constants (kept DVE-only so no cross-engine stall)
            xyz2 = sp.tile([128, 3 * 64], BF16)      # -2*xyz
            nc.vector.tensor_scalar_mul(xyz2[:], xyzt[:], -2.0)
            xsq = sp.tile([128, 3 * 64], F32)
            nc.vector.tensor_mul(xsq[:], xyzt[:], xyzt[:])
            x2a = sp.tile([128, 64], F32)
            nc.vector.tensor_add(x2a[:], xsq[:, 0:64], xsq[:, 64:128])
            x2s = sp.tile([128, 64], BF16)           # ||xyz||^2 per m
            nc.vector.tensor_add(x2s[:], x2a[:], xsq[:, 128:192])
            p2state["x2s"] = x2s

            # d = nx - 2*xyz ; cxsq = nx * d
            dt_ = wkp_.tile([128, 3 * S * 64], BF16, tag="w3k")
            d4 = dt_[:].rearrange("p (j s mi) -> p j s mi", j=3, s=S, mi=64)
            xyz2b = (
                xyz2[:]
                .rearrange("p (j one mi) -> p j one mi", j=3, one=1, mi=64)
                .broadcast_to((128, 3, S, 64))
            )
            nc.vector.tensor_add(d4, nx4, xyz2b)
            cs = wkp_.tile([128, 3 * S * 64], BF16, tag="w3k")
            cs4 = cs[:].rearrange("p (j s mi) -> p j s mi", j=3, s=S, mi=64)
            nc.vector.tensor_mul(cs4, nx4, d4)
            p2state["cs"] = cs

        def phase2b():
            # norm2 = sum_j cxsq + ||xyz||^2, clamped; norm = sqrt
            cs = p2state["cs"]
            n2a = wkp_.tile([128, 64 * S], BF16, tag="w1k")
            nc.vector.tensor_add(n2a[:], cs[:, 0:1024], cs[:, 1024:2048])
            n2b = wkp_.tile([128, 64 * S], BF16, tag="w1k")
            nc.vector.tensor_add(n2b[:], n2a[:], cs[:, 2048:3072])
            x2sb = (
                p2state["x2s"][:]
                .rearrange("p (one mi) -> p one mi", one=1)
                .broadcast_to((128, S, 64))
            )
            n2c = wkp_.tile([128, 64 * S], BF16, tag="w1k")
            nc.vector.tensor_add(
                n2c[:].rearrange("p (s mi) -> p s mi", s=S),
                n2b[:].rearrange("p (s mi) -> p s mi", s=S),
                x2sb,
            )
            n2d = wkp_.tile([128, 64 * S], BF16, tag="w1k")
            nc.vector.tensor_scalar_max(n2d[:], n2c[:], 0.0)
            nrm = wkp_.tile([128, 64 * S], BF16, tag="nrm")
            nc.scalar.sqrt(nrm[:], n2d[:])
            p2state["nrm"] = nrm

        def phase2c():
            # pl = nx * qp ; attn2 = qp3*norm - sum_j pl  (fp32)
            qpb = (
                qpt[:, 0:192]
                .rearrange("p (j one mi) -> p j one mi", j=3, one=1, mi=64)
                .broadcast_to((128, 3, S, 64))
            )
            pl = wkp_.tile([128, 3 * S * 64], BF16, tag="w3k")
            pl4 = pl[:].rearrange("p (j s mi) -> p j s mi", j=3, s=S, mi=64)
            nc.vector.tensor_mul(pl4, nx4, qpb)
            pla = wkp_.tile([128, 64 * S], BF16, tag="w1k")
            nc.vector.tensor_add(pla[:], pl[:, 0:1024], pl[:, 1024:2048])
            plb = wkp_.tile([128, 64 * S], BF16, tag="w1k")
            nc.vector.tensor_add(plb[:], pla[:], pl[:, 2048:3072])

            qp3b = (
                qpt[:, 192:256]
                .rearrange("p (one mi) -> p one mi", one=1)
                .broadcast_to((128, S, 64))
            )
            a2m = wkp_.tile([128, 64 * S], F32, tag="w1kf")
            nc.vector.tensor_mul(
                a2m[:].rearrange("p (s mi) -> p s mi", s=S),
                p2state["nrm"][:].rearrange("p (s mi) -> p s mi", s=S),
                qp3b,
            )
            plf = wkp_.tile([128, 64 * S], F32, tag="w1kf")
            nc.vector.tensor_copy(plf[:], plb[:])
            nc.vector.tensor_sub(attn2[:], a2m[:], plf[:])

        # ---- phase 3: main-term supertiles (qw chunk t computed just
        # ahead of its own supertile so reduce-0 starts early) ----
        for t in range(NT):
            if t >= 2:
                qw_chunk(t)
            npt = npts.pop(t)
            pieces = [(0, 8), (8, 16)] if t in (0, NT - 1) else [(0, 16)]
            for klo, khi in pieces:
                nmh = (khi - klo) // 2
                prod = prp.tile([128, (khi - klo) * 512], BF16, tag="prod",
                                name="prod")
                qwb = (
                    qw[:, t * 512 + (klo // 2) * 64:t * 512 + (khi // 2) * 64]
                    .rearrange("p (mh one ml) -> p mh one ml", mh=nmh, one=1, ml=64)
                    .broadcast_to((128, nmh, S, 64))
                )
                nc.vector.tensor_mul(
                    prod[:].rearrange("p (mh s ml) -> p mh s ml", mh=nmh, s=S, ml=64),
                    npt[:, klo * 512:khi * 512].rearrange(
                        "p (mh s ml) -> p mh s ml", mh=nmh, s=S, ml=64
                    ),
                    qwb,
                )

                ps = psm.tile([32, 512], F32, name="ps")
                for k in range(klo, khi):
                    nc.tensor.matmul(
                        ps[:],
                        lhsT=hs[:, k * 32:(k + 1) * 32],
                        rhs=prod[:, (k - klo) * 512:(k - klo + 1) * 512],
                        start=(k == klo),
                        stop=(k == khi - 1),
                    )
                sc = scp.tile([32, 512], F32, name="sc")
                nc.scalar.copy(sc[:], ps[:])
                # row h*16 + (mh*2+sh) holds cols (s8:8, ml:64) of dst
                # partition h*64 + t*8 + mh, col (sh*8+s8)*64 + ml. One
                # scatter per half, split across the two DMA-capable queues.
                for h, dma_eng in ((0, nc.gpsimd), (1, nc.scalar)):
                    dma_eng.dma_start(
                        attn1[
                            h * 64 + t * 8 + klo // 2:h * 64 + t * 8 + khi // 2, :
                        ].rearrange("p (sh s8 ml) -> p sh s8 ml", sh=2, s8=8, ml=64),
                        sc[h * 16 + klo:h * 16 + khi, :],
                    )
            if t + 4 < NT:
                issue_np(t + 4)
            if t == 1:
                phase2a()
            elif t == 2:
                phase2b()
            elif t == 3:
                phase2c()

        # ---- phase 4: softmax over s (no max-sub; |attn| < 4) ----
        attn = wkp_.tile([128, 64 * S], F32, tag="w1kf")
        nc.vector.tensor_add(attn[:], attn1[:], attn2[:])
        e = wkp_.tile([128, 64 * S], F32, tag="e")
        nc.scalar.activation(e[:], attn[:], AF.Exp)
        se = sp.tile([128, 64], F32)
        nc.vector.reduce_sum(
            se[:], e[:].rearrange("p (s mi) -> p mi s", s=S), axis=AX.X
        )
        rse = sp.tile([128, 64], F32)
        nc.vector.reciprocal(rse[:], se[:])
        o = wkp_.tile([128, 64 * S], BF16, tag="obf")
        rb = rse[:].rearrange("p (one mi) -> p one mi", one=1).broadcast_to((128, S, 64))
        nc.vector.tensor_mul(
            o[:].rearrange("p (s mi) -> p s mi", s=S),
            e[:].rearrange("p (s mi) -> p s mi", s=S),
            rb,
        )
        dma(OUT, o[:])


_NC_CACHE = None


def build_nc():
    global _NC_CACHE
    if _NC_CACHE is None:
        nc = bacc.Bacc(trn_type="TRN2", target_bir_lowering=False, debug=False)
        with tile.TileContext(nc) as tc:
            _body(tc)
        nc.compile()
        _NC_CACHE = nc
    return _NC_CACHE


def make_in_maps(xyz, neighbor_xyz, points, neighbor_points, Wk, Wpos, bpos):
    """Slice + relayout + bf16-cast full inputs into the 8 per-core maps."""
    xyz = np.asarray(xyz, dtype=np.float32)
    neighbor_xyz = np.asarray(neighbor_xyz, dtype=np.float32)
    points = np.asarray(points, dtype=np.float32)
    neighbor_points = np.asarray(neighbor_points, dtype=np.float32)
    Wk32 = np.asarray(Wk, dtype=np.float32)
    Wp32 = np.asarray(Wpos, dtype=np.float32)
    # one combined weight plane: [Wk | Wk^T | Wpos], duplicated onto both
    # partition halves
    wa1 = np.concatenate([Wk32, Wk32.T, Wp32], axis=1)  # [64, 132]
    WA = np.ascontiguousarray(np.concatenate([wa1, wa1], axis=0))  # [128, 132]

    in_maps = []
    for i in range(NCORES):
        nsl = slice(i * NL, (i + 1) * NL)
        # np: [B,C,nl,S] -> [c,m,s] -> [(h,c), (t, mh, s, ml)]
        npc = neighbor_points[:, :, nsl, :].transpose(1, 0, 2, 3).reshape(C, M, S)
        npd = (
            npc.reshape(C, 2, NT, 8, 64, S)
            .transpose(1, 0, 2, 3, 5, 4)
            .reshape(128, NT * TS)
        )
        # nx: [B,3,nl,S] -> [j,m,s] -> [p=m//64, (j, s, mi)]
        nxc = (
            neighbor_xyz[:, :, nsl, :]
            .transpose(1, 0, 2, 3)
            .reshape(3, M, S)
            .reshape(3, 128, 64, S)
            .transpose(1, 0, 3, 2)
            .reshape(128, 3 * S * 64)
        )
        # xyz: [B,3,nl] -> [p, (j, mi)]
        xc = (
            xyz[:, :, nsl]
            .transpose(1, 0, 2)
            .reshape(3, 128, 64)
            .transpose(1, 0, 2)
            .reshape(128, 192)
        )
        # points: [c, m] -> [(u, c), mm] full-width (u = m // MH)
        pc = points[:, :, nsl].transpose(1, 0, 2).reshape(C, M)
        pc = pc.reshape(C, 2, MH).transpose(1, 0, 2).reshape(128, MH)
        in_maps.append(
            {
                "NP": np.ascontiguousarray(npd.astype(NPBF)),
                "NX": np.ascontiguousarray(nxc.astype(NPBF)),
                "XYZ": np.ascontiguousarray(xc),
                "P": np.ascontiguousarray(pc.astype(NPBF)),
                "WA": WA,
            }
        )
    return in_maps


def assemble_output(results):
    """Per-core OUT [128, (s:16, mi:64)] -> full [B, N, S]."""
    out = np.empty((B, N, S), dtype=np.float32)
    for i in range(NCORES):
        oc = np.asarray(results[i]["OUT"]).astype(np.float32).reshape(128, S, 64)
        oc = oc.transpose(0, 2, 1).reshape(M, S)
        out[:, i * NL:(i + 1) * NL, :] = oc.reshape(B, NL, S)
    return out


def run_cores(in_maps, trace=False, trace_kwargs=None):
    nc = build_nc()
    return run_bass_kernel_spmd(
        nc,
        in_maps,
        core_ids=list(range(NCORES)),
        trace=trace,
        **(trace_kwargs or {}),
    )


def kernel(xyz, neighbor_xyz, points, neighbor_points, Wk, Wpos, bpos):
    in_maps = make_in_maps(
        xyz, neighbor_xyz, points, neighbor_points, Wk, Wpos, bpos
    )
    res = run_cores(in_maps, trace=False)
    return assemble_output(res.results)
